# revision 13
# baseline (speedup 1.0000x reference)
"""DiagonalLinear: y = x * w + b (elementwise over features).

x: (16384, 4096) f32, w/b: (4096,) f32.
Sharding: data-parallel over batch across 8 NeuronCores (2048 rows each),
weight/bias replicated.

Per core: HWDGE loads x row-tiles [128, Q*4096] into `tin`, DVE computes
mul+add into `tout` against partition-broadcast const tiles, HWDGE stores
`tout`. Separate in/out pools keep every tile at two actors so no
instruction needs more sync-waits than gen3 codegen allows (TensorTensor
<=2, HWDGE DMACopy <=1): loads WAR-wait only on DVE, stores RAW-wait only
on DVE, and the mul carries the load-lane + store-lane waits.

w and b are packed host-side into one [2, 4096] tensor so the constants
arrive in a single DMA; a tiny high-priority DVE read of that tile absorbs
the const dependency into the vector engine's clock.
"""

import numpy as np

import concourse.bacc as bacc
import concourse.bass as bass
import concourse.mybir as mybir
import concourse.tile as tile
from concourse.bass_utils import run_bass_kernel_spmd

N_CORES = 8
BATCH = 16384
D = 4096
ROWS_PER_CORE = BATCH // N_CORES  # 2048
P = 128

# Tunables
Q = 1          # 128-row blocks per SBUF tile -> tile free dim = Q*4096 (2 MiB DMAs)
BUFS = 4       # slots per pool (2 pools): 8*16KiB + 32KiB consts = 160KiB/partition

_CACHE = {}


def build_nc(q=Q, bufs=BUFS):
    nc = bacc.Bacc()
    f32 = mybir.dt.float32
    x = nc.dram_tensor("x", [ROWS_PER_CORE, D], f32, kind="ExternalInput")
    wb_in = nc.dram_tensor("wb", [1, 2 * D], f32, kind="ExternalInput")
    y = nc.dram_tensor("y", [ROWS_PER_CORE, D], f32, kind="ExternalOutput")

    n_tiles = ROWS_PER_CORE // (P * q)
    assert n_tiles * P * q == ROWS_PER_CORE

    # tile n, partition p, free (j, d) <-> row n*(q*P) + j*P + p, col d
    x_r = x.rearrange("(n j p) d -> n p j d", p=P, j=q)
    y_r = y.rearrange("(n j p) d -> n p j d", p=P, j=q)

    with tile.TileContext(nc) as tc:
        with (
            tc.tile_pool(name="consts", bufs=1) as cpool,
            tc.tile_pool(name="tin", bufs=bufs) as pin,
            tc.tile_pool(name="tout", bufs=bufs) as pout,
        ):
            consts = cpool.tile([P, 2 * D], f32)  # [:, :D]=w, [:, D:]=b
            rows = cpool.tile([1, 2 * D], f32)
            scratch = cpool.tile([P, 1], f32)
            with tc.high_priority():
                # 32 KiB DMA + on-chip GpSimd broadcast keeps the const
                # replication off the (saturated) SDMA fabric.
                nc.sync.dma_start(rows[:, :], wb_in[:, :])
                nc.gpsimd.partition_broadcast(consts[:, 0:D], rows[0:1, 0:D])
                nc.gpsimd.partition_broadcast(consts[:, D : 2 * D], rows[0:1, D : 2 * D])
                # absorb the const dep into DVE's clock
                nc.vector.tensor_copy(scratch[:, :], consts[:, 0:1])
                nc.vector.tensor_copy(scratch[:, :], consts[:, D : D + 1])

            wt = consts[:, 0:D]
            bt = consts[:, D : 2 * D]
            for i in range(n_tiles):
                tin = pin.tile([P, q * D], f32)
                tout = pout.tile([P, q * D], f32)
                nc.sync.dma_start(tin[:, :].rearrange("p (j d) -> p j d", j=q), x_r[i])
                for j in range(q):
                    sl = slice(j * D, (j + 1) * D)
                    nc.vector.tensor_mul(tout[:, sl], tin[:, sl], wt)
                    nc.vector.tensor_add(tout[:, sl], tout[:, sl], bt)
                nc.scalar.dma_start(y_r[i], tout[:, :].rearrange("p (j d) -> p j d", j=q))
    nc.compile()
    return nc


def _get_nc():
    if "nc" not in _CACHE:
        _CACHE["nc"] = build_nc()
    return _CACHE["nc"]


def run(input, weight, bias, nc=None, **spmd_kwargs):
    if nc is None:
        nc = _get_nc()
    x = np.ascontiguousarray(input, dtype=np.float32)
    wb = np.ascontiguousarray(
        np.stack([np.asarray(weight), np.asarray(bias)]).astype(np.float32)
    ).reshape(1, 2 * D)
    in_maps = [
        {"x": x[c * ROWS_PER_CORE : (c + 1) * ROWS_PER_CORE], "wb": wb}
        for c in range(N_CORES)
    ]
    res = run_bass_kernel_spmd(nc, in_maps, core_ids=list(range(N_CORES)), **spmd_kwargs)
    out = np.concatenate([r["y"] for r in res.results], axis=0)
    return out, res


def kernel(input, weight, bias):
    out, _ = run(input, weight, bias)
    return out
